# revision 1
# baseline (speedup 1.0000x reference)
"""Trainium2 Bass kernel for CalculateDirectionFeature.

Computes V[b,n,f,t] = sum_p cos(obs_ipd[b,p,f,t] - tpd[b,p,n,f]) where
tpd = 2*pi*freq[f] * (pair_vec[p] . r[b,n]) / v_sound.

Strategy (fp16 end-to-end, memory-regime kernel):
  cos(a-b) = cos(a)cos(b) + sin(a)sin(b) turns the pair-reduction into a
  small matmul contracting over (trig, pair) = 12 rows per frequency bin.
  The host precomputes BOTH trig factors (no on-device activations):
    rhs  marr[(cs,p,g), t] = trig_cs(obs[b, p, f(j,g), t])   (fp16)
    lhsT wts[(cs,p,g), (n,g')] = delta_gg' * trig_cs(tpd[b,p,n,f(j,g)])
  Sharding: 8 cores = 4 batches x 2 FREQUENCY halves of 129 bins each
  (257 = 129 + 129 - 1 overlap) — splitting freq instead of dirs halves
  the dominant obs-trig input per core.  Frequencies are packed G=3 per
  matmul (block-diagonal weights): K = 2*6*3 = 36 contraction rows,
  M = 36 dirs * 3 freqs = 108 psum partitions, N = 300 time steps;
  43 matmuls cover the 129 bins with no padding.

  DMA layout (from measured ring behavior): the software DGE ring
  (gpsimd) moves DRAM->SBUF fastest (~220 GB/s); input rides it as
  stage-ordered fused (wts_s | marr_s) chunks, each <=4.8KB/partition
  and 64B-aligned with pad gaps (bigger single-partition packets halve
  the per-engine DMA rate).  Output: half of every stage on a HW ring
  (sync/scalar alternating), half on the SW ring behind the input.

  Each PSUM pair (2 banks / 2 matmuls) is staged to fp16 SBUF by TWO
  copies in parallel — Vector takes t[0:150), Scalar takes t[150:300) —
  halving the psum-recycle latency so the Tensor engine is never
  throttled by copy latency.  Host upcasts fp16 to fp32.
"""

import numpy as np

B, P, NQ, F, T = 4, 6, 36, 257, 300
V_SOUND = 343.0
G = 3                # freq bins per matmul
M = NQ * G           # 108 psum partitions per matmul
K = 2 * P * G        # 36 contraction rows (cs, p, g)
NJ = 43              # matmuls per core
FPC = NJ * G         # 129 freq bins per core (257 = 129 + 129 - 1 overlap)
FBASE = [0, F - FPC]           # per-half first global freq bin (0, 122)
TH = T // 2          # copy split point (150)
STAGE_Q = [5, 8, 8, 8, 8, 6]   # matmuls per output stage
STAGE_J0 = [0, 5, 13, 21, 29, 37]
STAGE_F0 = [0, 15, 39, 63, 87, 111]   # local freq offset of each stage
NS = len(STAGE_Q)


def _r32(x):
    return ((x + 31) // 32) * 32


# fused per-stage column layout: [wts_s | pad | marr_s | pad], every chunk
# start 32-col (64B) aligned, identical in DRAM and SBUF
WOFF, MOFF = [], []
_acc = 0
for _q in STAGE_Q:
    WOFF.append(_acc)
    _acc = _r32(_acc + _q * M)
    MOFF.append(_acc)
    _acc = _r32(_acc + _q * T)
NCOL = _acc

LAST_RESULTS = None
_cache = {}


def _fmap():
    """fmap[j, g]: frequency bin computed by matmul j, group position g."""
    fm = np.empty((NJ, G), np.int64)
    for q_, f0, j0 in zip(STAGE_Q, STAGE_F0, STAGE_J0):
        for q in range(q_):
            for g in range(G):
                fm[j0 + q, g] = f0 + q_ * g + q
    return fm


def _build_nc():
    import concourse.bacc as bacc
    import concourse.tile as tile
    import concourse.mybir as mybir

    f16 = mybir.dt.float16
    f32 = mybir.dt.float32

    nc = bacc.Bacc(
        "TRN2",
        target_bir_lowering=False,
        debug=False,
        enable_asserts=False,
        num_devices=8,
    )
    inp_d = nc.dram_tensor("inp", [K, NCOL], f16, kind="ExternalInput").ap()
    out_d = nc.dram_tensor("out", [NQ, FPC, T], f16, kind="ExternalOutput").ap()

    with tile.TileContext(nc) as tc:
        with (
            tc.tile_pool(name="io", bufs=1) as io,
            tc.tile_pool(name="psum", bufs=4, space="PSUM") as psum,
            tc.tile_pool(name="stage", bufs=6) as stage,
        ):
            inp = io.tile([K, NCOL], f16)
            scr = io.tile([4, 16], f16)

            # 4-byte warm-up DMAs: wake the cold HW DGE rings early so the
            # first real output DMA doesn't pay the ring cold-start.
            nc.sync.dma_start(out=scr[0:1, 0:2], in_=inp_d[0:1, 0:2])
            nc.scalar.dma_start(out=scr[1:2, 0:2], in_=inp_d[0:1, 0:2])

            # PE p-state warm-up: the PE only reaches 2.4 GHz after ~3us of
            # continuous execution, and it idles from the 7.2us preamble
            # barrier until stage-0 data lands (~11.3us).  Chain dummy
            # matmuls on scratch data through that window so real matmuls
            # start at full clock instead of 1.2 GHz.
            warm = io.tile([K, 512], f16)
            nc.vector.memset(warm[:, :], 0.0)
            for wv in range(8):
                ptw = psum.tile(
                    [M, 2, 512], f32, tag="pt", name=f"pt{wv % 4}"
                )
                nc.tensor.matmul(
                    ptw[:, 0, 0:512],
                    lhsT=warm[:, 0:M],
                    rhs=warm[:, 0:512],
                    start=True,
                    stop=True,
                )

            # input on the gpsimd SW ring, FIFO in stage need-order
            for s in range(NS):
                w0, w1 = WOFF[s], WOFF[s] + STAGE_Q[s] * M
                m0, m1 = MOFF[s], MOFF[s] + STAGE_Q[s] * T
                nc.gpsimd.dma_start(out=inp[:, w0:w1], in_=inp_d[:, w0:w1])
                nc.gpsimd.dma_start(out=inp[:, m0:m1], in_=inp_d[:, m0:m1])

            for s, (q_, f0, j0) in enumerate(
                zip(STAGE_Q, STAGE_F0, STAGE_J0)
            ):
                st = stage.tile([M, q_, T], f16, tag="st", name=f"st{s}")
                q = 0
                pcnt = 0
                while q < q_:
                    w = min(2, q_ - q)
                    pt = psum.tile(
                        [M, 2, 512],
                        f32,
                        tag="pt",
                        name=f"pt{(STAGE_J0[s] // 2 + pcnt) % 4}",
                    )
                    for slot in range(w):
                        nc.tensor.matmul(
                            pt[:, slot, 0:T],
                            lhsT=inp[
                                :,
                                WOFF[s] + (q + slot) * M : WOFF[s]
                                + (q + slot + 1) * M,
                            ],
                            rhs=inp[
                                :,
                                MOFF[s] + (q + slot) * T : MOFF[s]
                                + (q + slot + 1) * T,
                            ],
                            start=True,
                            stop=True,
                        )
                    # both engines stage this pair in parallel (t halves)
                    nc.vector.tensor_copy(
                        out=st[:, q : q + w, 0:TH],
                        in_=pt[:, 0:w, 0:TH],
                    )
                    nc.scalar.copy(
                        out=st[:, q : q + w, TH:T],
                        in_=pt[:, 0:w, TH:T],
                    )
                    pcnt += 1
                    q += w

                def odst(n0, n1):
                    return out_d[n0:n1, f0 : f0 + G * q_, :].rearrange(
                        "n (g q) t -> n g (q t)", q=q_
                    )

                # half of every stage on the sync HW ring (issued from the
                # otherwise-idle sync sequencer so the scalar engine's copy
                # stream is never interrupted by ~850ns DMA-issue stalls),
                # half on the SW ring behind the input stream.
                nc.sync.dma_start(out=odst(0, 18), in_=st[0:54, :, :])
                nc.gpsimd.dma_start(out=odst(18, NQ), in_=st[54:108, :, :])
    nc.compile()
    return nc


def _get_nc():
    if "nc" not in _cache:
        _cache["nc"] = _build_nc()
    return _cache["nc"]


def _prep_inputs(observed_ipd, query_azi, query_ele, pair_vectors, freq_bins):
    obs = np.asarray(observed_ipd, np.float64).reshape(B, P, F, T)
    azi = np.asarray(query_azi, np.float64)
    ele = np.asarray(query_ele, np.float64)
    pv = np.asarray(pair_vectors, np.float64)
    fb = np.asarray(freq_bins, np.float64)
    fm = _fmap()

    se, ce = np.sin(ele), np.cos(ele)
    r = np.stack([se * np.cos(azi), se * np.sin(azi), ce], axis=1)  # (B,3,NQ)
    tdoa = np.einsum("pc,bcn->bpn", pv, r) / V_SOUND  # (B,P,NQ)
    tpd = 2.0 * np.pi * tdoa[..., None] * fb  # (B,P,NQ,F)
    wtrig = np.stack([np.cos(tpd), np.sin(tpd)], axis=0)  # (2,B,P,NQ,F)

    in_maps = []
    for b in range(B):
        ct, st_ = np.cos(obs[b]), np.sin(obs[b])  # (P,F,T)
        for h in range(2):
            gf = FBASE[h] + fm                    # (NJ,G) global bins
            ma = np.stack([ct[:, gf, :], st_[:, gf, :]])   # (2,P,NJ,G,T)
            ma = ma.transpose(0, 1, 3, 2, 4).reshape(K, NJ, T)
            wt = wtrig[:, b]                      # (2,P,NQ,F)
            wfull = np.zeros((2, P, G, NJ, NQ, G), np.float64)
            for g in range(G):
                sel = wt[:, :, :, gf[:, g]]       # (2,P,NQ,NJ)
                wfull[:, :, g, :, :, g] = sel.transpose(0, 1, 3, 2)
            wts = wfull.reshape(K, NJ, M)
            inp = np.zeros((K, NCOL), np.float16)
            for s, (q_, j0) in enumerate(zip(STAGE_Q, STAGE_J0)):
                inp[:, WOFF[s] : WOFF[s] + q_ * M] = wts[
                    :, j0 : j0 + q_, :
                ].reshape(K, q_ * M)
                inp[:, MOFF[s] : MOFF[s] + q_ * T] = ma[
                    :, j0 : j0 + q_, :
                ].reshape(K, q_ * T)
            in_maps.append({"inp": np.ascontiguousarray(inp)})
    return in_maps


def kernel(observed_ipd, query_azi, query_ele, pair_vectors, freq_bins):
    global LAST_RESULTS
    from concourse.bass_utils import run_bass_kernel_spmd

    nc = _get_nc()
    in_maps = _prep_inputs(
        observed_ipd, query_azi, query_ele, pair_vectors, freq_bins
    )
    res = run_bass_kernel_spmd(nc, in_maps, core_ids=list(range(8)))
    LAST_RESULTS = res
    out = np.empty((B, NQ, F, T), np.float32)
    for c in range(8):
        b, h = divmod(c, 2)
        o = res.results[c]["out"].astype(np.float32)  # (36, FPC, T)
        if h == 0:
            out[b, :, :FPC] = o
        else:
            out[b, :, FPC:] = o[:, 2 * FPC - F :, :]
    return out



# revision 2
# speedup vs baseline: 1.1296x; 1.1296x over previous
"""Trainium2 Bass kernel for CalculateDirectionFeature.

V[b,n,f,t] = sum_p cos(obs_ipd[b,p,f,t] - tpd[b,p,n,f]);
cos(a-b) = cos(a)cos(b) + sin(a)sin(b) -> matmul contracting
K = (2 trig x 6 pairs x G=3 freqs) = 36, M = 108 (36 dirs x 3 freqs),
N = 300 (time).  Host precomputes both trig factors; 8 cores =
4 batches x 2 freq halves (129 bins each, 1 overlapping bin).

PE: 64x128 two-tile mode — psum group u computes j=2u on row-tile T0
(operands at SBUF partitions 0-35) and j=2u+1 on T8 (partitions
64-99).  The split's real win is the input DMA: each stream's
partitions map onto a distinct (even/odd) half of the 16 SDMA engines.

PSUM: [128, 2banks, 512] tile per group, 4 rotating buffers (all 8
banks).  Evacuation alternates whole groups between DVE (even u) and
ACT (odd u) into separate staging tiles — exactly one reader per psum
buffer keeps the Tile sem assigner from chaining cross-engine waits,
and 4-deep rotation rides out its conservative (+2 group) wait ticks.

Input rides stage-chunked DMAs: stage 0 on the sync HWDGE ring (low
first-byte latency), the rest on the gpsimd SWDGE ring (fastest for
DRAM->SBUF).  Output: flat [108, 12900] fp16 in (engine-region, group,
slot, t) order, two DMAs per output group alternating sync/gpsimd
queues; the host unscrambles and upcasts.
"""

import numpy as np

B, P, NQ, F, T = 4, 6, 36, 257, 300
V_SOUND = 343.0
G = 3
M = NQ * G           # 108
K = 2 * P * G        # 36
NJ = 43
FPC = NJ * G         # 129
FBASE = [0, F - FPC]

CHUNK = 416          # 108 wts + 300 marr + 8 pad
WOFS, MOFS = 0, 108
NT0 = (NJ + 1) // 2  # 22
NT8 = NJ // 2        # 21
NCOL = NT0 * CHUNK   # 9152

NPG = 22             # psum groups u: j = {2u, 2u+1} (u=21: j=42 only)
NOG = 6              # output groups of 4 psum groups
VOFS = 11 * 600      # out cols [0,VOFS) = DVE (even u) groups, rest ACT

# input stages as psum-group ranges
STAGES = [(0, 2), (2, 6), (6, 13), (13, NPG)]
# output groups as psum-group ranges
OGS = [(0, 6), (6, 12), (12, 18), (18, NPG)]
OG_ENG = ["sync", "gpsimd", "sync", "gpsimd"]


def _stage_cols(s):
    ulo, uhi = STAGES[s]
    return ulo * CHUNK, uhi * CHUNK


_cache = {}
LAST_RESULTS = None


def _build_nc():
    import concourse.bacc as bacc
    import concourse.tile as tile
    import concourse.mybir as mybir

    f16 = mybir.dt.float16
    f32 = mybir.dt.float32

    nc = bacc.Bacc(
        "TRN2",
        target_bir_lowering=False,
        debug=False,
        enable_asserts=False,
        num_devices=8,
    )
    # rows 0-35: T0 stream, rows 64-99: T8 stream (rows 36-63 pad so each
    # stage moves as ONE rectangle; the pad rides otherwise-idle SDMA
    # engines).
    inp_d = nc.dram_tensor("inp", [100, NCOL], f16, kind="ExternalInput").ap()
    # columns [0, VOFS) = T0 (DVE) outputs by q, rest = T8 (ACT) by q
    out_d = nc.dram_tensor("out", [M, NJ * 300], f16, kind="ExternalOutput").ap()

    with tile.TileContext(nc) as tc:
        with (
            tc.tile_pool(name="io", bufs=1) as io,
            tc.tile_pool(name="psum", bufs=4, space="PSUM") as psum,
            tc.tile_pool(name="stage", bufs=3) as stage,
        ):
            inp = io.tile([128, NCOL], f16)
            for s in range(len(STAGES)):
                a0, a1 = _stage_cols(s)
                eng = nc.sync if s == 0 else nc.gpsimd
                eng.dma_start(out=inp[0:K, a0:a1], in_=inp_d[0:K, a0:a1])
                eng.dma_start(
                    out=inp[64:64 + K, a0:a1], in_=inp_d[64:64 + K, a0:a1]
                )

            stv = sta = None
            ogi = 0
            svc = sac = 0          # slot counters within current og tiles
            v0 = a0_ = 0           # first vidx/aidx of current og
            for u in range(NPG):
                njs = 2 if 2 * u + 1 < NJ else 1   # j = 2u (T0), 2u+1 (T8)
                pt = psum.tile([128, 2, 512], f32, tag="pt", name=f"pt{u % 4}")
                for s in range(njs):
                    j = 2 * u + s
                    base = 0 if s == 0 else 64
                    c = u * CHUNK
                    nc.tensor.matmul(
                        pt[0:M, s, 0:300],
                        lhsT=inp[base:base + K, c + WOFS:c + WOFS + M],
                        rhs=inp[base:base + K, c + MOFS:c + MOFS + 300],
                        start=True, stop=True,
                    )
                if u == OGS[ogi][0]:
                    stv = stage.tile([M, 6, 300], f16, tag="stv",
                                     name=f"stv{ogi}", bufs=2)
                    sta = stage.tile([M, 6, 300], f16, tag="sta",
                                     name=f"sta{ogi}", bufs=2)
                    svc = sac = 0
                    v0, a0_ = u // 2, u // 2
                if u % 2 == 0:
                    nc.vector.tensor_copy(
                        out=stv[0:M, svc:svc + njs, :],
                        in_=pt[0:M, 0:njs, 0:300]
                    )
                    svc += njs
                else:
                    nc.scalar.copy(
                        out=sta[0:M, sac:sac + njs, :],
                        in_=pt[0:M, 0:njs, 0:300]
                    )
                    sac += njs
                if u == OGS[ogi][1] - 1:
                    dv = out_d[0:M, 600 * v0:600 * v0 + 300 * svc]
                    da = out_d[0:M, VOFS + 600 * a0_:
                               VOFS + 600 * a0_ + 300 * sac]
                    eng = getattr(nc, OG_ENG[ogi])
                    eng.dma_start(out=dv, in_=stv[0:M, 0:svc, :])
                    eng.dma_start(out=da, in_=sta[0:M, 0:sac, :])
                    ogi += 1
    nc.compile()
    return nc


def _get_nc():
    if "nc" not in _cache:
        _cache["nc"] = _build_nc()
    return _cache["nc"]


def _prep_inputs(observed_ipd, query_azi, query_ele, pair_vectors, freq_bins):
    obs = np.asarray(observed_ipd, np.float64).reshape(B, P, F, T)
    azi = np.asarray(query_azi, np.float64)
    ele = np.asarray(query_ele, np.float64)
    pv = np.asarray(pair_vectors, np.float64)
    fb = np.asarray(freq_bins, np.float64)

    se, ce = np.sin(ele), np.cos(ele)
    r = np.stack([se * np.cos(azi), se * np.sin(azi), ce], axis=1)  # (B,3,NQ)
    tdoa = np.einsum("pc,bcn->bpn", pv, r) / V_SOUND
    tpd = 2.0 * np.pi * tdoa[..., None] * fb  # (B,P,NQ,F)
    wtrig = np.stack([np.cos(tpd), np.sin(tpd)])  # (2,B,P,NQ,F)

    in_maps = []
    for b in range(B):
        otrig = np.stack([np.cos(obs[b]), np.sin(obs[b])])  # (2,P,F,T)
        for h in range(2):
            inp = np.zeros((100, NCOL), np.float16)
            for j in range(NJ):
                gf = FBASE[h] + 3 * j + np.arange(G)
                row = 0 if j % 2 == 0 else 64
                q = j // 2
                c = q * CHUNK
                ma = otrig[:, :, gf, :]               # (2,P,G,T)
                inp[row:row + K, c + MOFS:c + MOFS + 300] = ma.reshape(K, T)
                wsel = wtrig[:, b][:, :, :, gf]       # (2,P,NQ,G)
                wfull = np.zeros((2, P, G, NQ, G))
                for g in range(G):
                    wfull[:, :, g, :, g] = wsel[:, :, :, g]
                inp[row:row + K, c + WOFS:c + WOFS + M] = wfull.reshape(K, M)
            in_maps.append({"inp": np.ascontiguousarray(inp)})
    return in_maps


def _unscramble(o):
    """o: [108, 12900] fp16 -> V_half [NQ, FPC, T] fp32."""
    v = np.empty((NQ, FPC, T), np.float32)
    of = o.astype(np.float32)
    for j in range(NJ):
        u = j // 2
        if u % 2 == 0:
            c = 600 * (u // 2) + 300 * (j % 2)
        else:
            c = VOFS + 600 * (u // 2) + 300 * (j % 2)
        v[:, 3 * j:3 * j + G, :] = of[:, c:c + 300].reshape(NQ, G, T)
    return v


def kernel(observed_ipd, query_azi, query_ele, pair_vectors, freq_bins):
    global LAST_RESULTS
    from concourse.bass_utils import run_bass_kernel_spmd

    nc = _get_nc()
    in_maps = _prep_inputs(
        observed_ipd, query_azi, query_ele, pair_vectors, freq_bins
    )
    res = run_bass_kernel_spmd(nc, in_maps, core_ids=list(range(8)))
    LAST_RESULTS = res
    out = np.empty((B, NQ, F, T), np.float32)
    for c in range(8):
        b, h = divmod(c, 2)
        v = _unscramble(res.results[c]["out"])
        out[b, :, FBASE[h]:FBASE[h] + FPC, :] = v
    return out


# revision 3
# speedup vs baseline: 1.1571x; 1.0244x over previous
"""Trainium2 Bass kernel for CalculateDirectionFeature.

V[b,n,f,t] = sum_p cos(obs_ipd[b,p,f,t] - tpd[b,p,n,f]);
cos(a-b) = cos(a)cos(b) + sin(a)sin(b) -> matmul contracting
K = (2 trig x 6 pairs x G=3 freqs) = 36, M = 108 (36 dirs x 3 freqs),
N = 300 (time).  Host precomputes both trig factors; 8 cores =
4 batches x 2 freq halves (129 bins each, 1 overlapping bin).

PE: 64x128 two-tile mode — psum group u computes j=2u on row-tile T0
(operands at SBUF partitions 0-35) and j=2u+1 on T8 (partitions
64-99).  The split's main win is the input DMA: each stream's
partitions map onto a distinct (even/odd) half of the 16 SDMA engines.

PSUM: [128, 2banks, 512] tile per group, 4 rotating buffers (all 8
banks).  Evacuation alternates whole groups between DVE (even u) and
ACT (odd u) into separate staging tiles — exactly one reader per psum
buffer keeps the Tile sem assigner from chaining cross-engine waits,
and the 4-deep rotation rides out its conservative (+2 group) ticks.

Input: stage 0 split across the sync+scalar HWDGE rings (parallel
issue, low first-byte latency), later stages on the gpsimd SWDGE ring
(fastest for bulk DRAM->SBUF).  Output: flat [108, 12900] fp16 in
(engine-region, group, slot, t) order; middle output groups ride the
gpsimd ring, first/last ride sync (HWDGE's shorter completion receipt
shortens the final drain).  Host unscrambles and upcasts.
"""

import numpy as np

B, P, NQ, F, T = 4, 6, 36, 257, 300
V_SOUND = 343.0
G = 3
M = NQ * G           # 108
K = 2 * P * G        # 36
NJ = 43
FPC = NJ * G         # 129
FBASE = [0, F - FPC]

CHUNK = 416          # 108 wts + 300 marr + 8 pad
WOFS, MOFS = 0, 108
NT0 = (NJ + 1) // 2  # 22
NT8 = NJ // 2        # 21
NCOL = NT0 * CHUNK   # 9152

NPG = 22             # psum groups u: j = {2u, 2u+1} (u=21: j=42 only)
NOG = 6              # output groups of 4 psum groups
VOFS = 11 * 600      # out cols [0,VOFS) = DVE (even u) groups, rest ACT

# input stages as psum-group ranges
STAGES = [(0, 2), (2, 6), (6, 13), (13, NPG)]
# output groups as psum-group ranges
OGS = [(0, 6), (6, 12), (12, 18), (18, NPG)]
OG_ENG = ["sync", "gpsimd", "gpsimd", "sync"]


def _stage_cols(s):
    ulo, uhi = STAGES[s]
    return ulo * CHUNK, uhi * CHUNK


_cache = {}
LAST_RESULTS = None


def _build_nc():
    import concourse.bacc as bacc
    import concourse.tile as tile
    import concourse.mybir as mybir

    f16 = mybir.dt.float16
    f32 = mybir.dt.float32

    nc = bacc.Bacc(
        "TRN2",
        target_bir_lowering=False,
        debug=False,
        enable_asserts=False,
        num_devices=8,
    )
    # rows 0-35: T0 stream, rows 64-99: T8 stream (rows 36-63 pad so each
    # stage moves as ONE rectangle; the pad rides otherwise-idle SDMA
    # engines).
    inp_d = nc.dram_tensor("inp", [100, NCOL], f16, kind="ExternalInput").ap()
    # columns [0, VOFS) = T0 (DVE) outputs by q, rest = T8 (ACT) by q
    out_d = nc.dram_tensor("out", [M, NJ * 300], f16, kind="ExternalOutput").ap()

    with tile.TileContext(nc) as tc:
        with (
            tc.tile_pool(name="io", bufs=1) as io,
            tc.tile_pool(name="psum", bufs=4, space="PSUM") as psum,
            tc.tile_pool(name="stage", bufs=3) as stage,
        ):
            inp = io.tile([128, NCOL], f16)
            for s in range(len(STAGES)):
                a0, a1 = _stage_cols(s)
                eng = nc.sync if s == 0 else nc.gpsimd
                eng2 = nc.scalar if s == 0 else nc.gpsimd
                eng.dma_start(out=inp[0:K, a0:a1], in_=inp_d[0:K, a0:a1])
                eng2.dma_start(
                    out=inp[64:64 + K, a0:a1], in_=inp_d[64:64 + K, a0:a1]
                )

            stv = sta = None
            ogi = 0
            svc = sac = 0          # slot counters within current og tiles
            v0 = a0_ = 0           # first vidx/aidx of current og
            for u in range(NPG):
                njs = 2 if 2 * u + 1 < NJ else 1   # j = 2u (T0), 2u+1 (T8)
                pt = psum.tile([128, 2, 512], f32, tag="pt", name=f"pt{u % 4}")
                for s in range(njs):
                    j = 2 * u + s
                    base = 0 if s == 0 else 64
                    c = u * CHUNK
                    nc.tensor.matmul(
                        pt[0:M, s, 0:300],
                        lhsT=inp[base:base + K, c + WOFS:c + WOFS + M],
                        rhs=inp[base:base + K, c + MOFS:c + MOFS + 300],
                        start=True, stop=True,
                    )
                if u == OGS[ogi][0]:
                    stv = stage.tile([M, 6, 300], f16, tag="stv",
                                     name=f"stv{ogi}", bufs=2)
                    sta = stage.tile([M, 6, 300], f16, tag="sta",
                                     name=f"sta{ogi}", bufs=2)
                    svc = sac = 0
                    v0, a0_ = (u + 1) // 2, u // 2
                if u % 2 == 0:
                    nc.vector.tensor_copy(
                        out=stv[0:M, svc:svc + njs, :],
                        in_=pt[0:M, 0:njs, 0:300]
                    )
                    svc += njs
                else:
                    nc.scalar.copy(
                        out=sta[0:M, sac:sac + njs, :],
                        in_=pt[0:M, 0:njs, 0:300]
                    )
                    sac += njs
                if u == OGS[ogi][1] - 1:
                    dv = out_d[0:M, 600 * v0:600 * v0 + 300 * svc]
                    da = out_d[0:M, VOFS + 600 * a0_:
                               VOFS + 600 * a0_ + 300 * sac]
                    eng = getattr(nc, OG_ENG[ogi])
                    eng.dma_start(out=dv, in_=stv[0:M, 0:svc, :])
                    eng.dma_start(out=da, in_=sta[0:M, 0:sac, :])
                    ogi += 1
    nc.compile()
    return nc


def _get_nc():
    if "nc" not in _cache:
        _cache["nc"] = _build_nc()
    return _cache["nc"]


def _prep_inputs(observed_ipd, query_azi, query_ele, pair_vectors, freq_bins):
    obs = np.asarray(observed_ipd, np.float64).reshape(B, P, F, T)
    azi = np.asarray(query_azi, np.float64)
    ele = np.asarray(query_ele, np.float64)
    pv = np.asarray(pair_vectors, np.float64)
    fb = np.asarray(freq_bins, np.float64)

    se, ce = np.sin(ele), np.cos(ele)
    r = np.stack([se * np.cos(azi), se * np.sin(azi), ce], axis=1)  # (B,3,NQ)
    tdoa = np.einsum("pc,bcn->bpn", pv, r) / V_SOUND
    tpd = 2.0 * np.pi * tdoa[..., None] * fb  # (B,P,NQ,F)
    wtrig = np.stack([np.cos(tpd), np.sin(tpd)])  # (2,B,P,NQ,F)

    in_maps = []
    for b in range(B):
        otrig = np.stack([np.cos(obs[b]), np.sin(obs[b])])  # (2,P,F,T)
        for h in range(2):
            inp = np.zeros((100, NCOL), np.float16)
            for j in range(NJ):
                gf = FBASE[h] + 3 * j + np.arange(G)
                row = 0 if j % 2 == 0 else 64
                q = j // 2
                c = q * CHUNK
                ma = otrig[:, :, gf, :]               # (2,P,G,T)
                inp[row:row + K, c + MOFS:c + MOFS + 300] = ma.reshape(K, T)
                wsel = wtrig[:, b][:, :, :, gf]       # (2,P,NQ,G)
                wfull = np.zeros((2, P, G, NQ, G))
                for g in range(G):
                    wfull[:, :, g, :, g] = wsel[:, :, :, g]
                inp[row:row + K, c + WOFS:c + WOFS + M] = wfull.reshape(K, M)
            in_maps.append({"inp": np.ascontiguousarray(inp)})
    return in_maps


def _unscramble(o):
    """o: [108, 12900] fp16 -> V_half [NQ, FPC, T] fp32."""
    v = np.empty((NQ, FPC, T), np.float32)
    of = o.astype(np.float32)
    for j in range(NJ):
        u = j // 2
        if u % 2 == 0:
            c = 600 * (u // 2) + 300 * (j % 2)
        else:
            c = VOFS + 600 * (u // 2) + 300 * (j % 2)
        v[:, 3 * j:3 * j + G, :] = of[:, c:c + 300].reshape(NQ, G, T)
    return v


def kernel(observed_ipd, query_azi, query_ele, pair_vectors, freq_bins):
    global LAST_RESULTS
    from concourse.bass_utils import run_bass_kernel_spmd

    nc = _get_nc()
    in_maps = _prep_inputs(
        observed_ipd, query_azi, query_ele, pair_vectors, freq_bins
    )
    res = run_bass_kernel_spmd(nc, in_maps, core_ids=list(range(8)))
    LAST_RESULTS = res
    out = np.empty((B, NQ, F, T), np.float32)
    for c in range(8):
        b, h = divmod(c, 2)
        v = _unscramble(res.results[c]["out"])
        out[b, :, FBASE[h]:FBASE[h] + FPC, :] = v
    return out


# revision 4
# speedup vs baseline: 1.1634x; 1.0055x over previous
"""Trainium2 Bass kernel for CalculateDirectionFeature.

V[b,n,f,t] = sum_p cos(obs_ipd[b,p,f,t] - tpd[b,p,n,f]);
cos(a-b) = cos(a)cos(b) + sin(a)sin(b) -> matmul contracting
K = (2 trig x 6 pairs x G=3 freqs) = 36, M = 108 (36 dirs x 3 freqs),
N = 300 (time).  Host precomputes both trig factors; 8 cores =
4 batches x 2 freq halves (129 bins each, 1 overlapping bin).

PE: 64x128 two-tile mode — psum group u computes j=2u on row-tile T0
(operands at SBUF partitions 0-35) and j=2u+1 on T8 (partitions
64-99).  The split's main win is the input DMA: each stream's
partitions map onto a distinct (even/odd) half of the 16 SDMA engines.

PSUM: [128, 2banks, 512] tile per group, 4 rotating buffers (all 8
banks).  Evacuation alternates whole groups between DVE (even u) and
ACT (odd u) into separate staging tiles — exactly one reader per psum
buffer keeps the Tile sem assigner from chaining cross-engine waits,
and the 4-deep rotation rides out its conservative (+2 group) ticks.

Input: stage 0 split across the sync+scalar HWDGE rings (parallel
issue, low first-byte latency), later stages on the gpsimd SWDGE ring
(fastest for bulk DRAM->SBUF).  Output: flat [108, 12900] fp16 in
(engine-region, group, slot, t) order; middle output groups ride the
gpsimd ring, the first and the two small tail groups ride the sync and
scalar HWDGE rings (shorter completion receipt, and the scalar ring
issues the final pair right after its last copy).  Host unscrambles
and upcasts.
"""

import numpy as np

B, P, NQ, F, T = 4, 6, 36, 257, 300
V_SOUND = 343.0
G = 3
M = NQ * G           # 108
K = 2 * P * G        # 36
NJ = 43
FPC = NJ * G         # 129
FBASE = [0, F - FPC]

CHUNK = 416          # 108 wts + 300 marr + 8 pad
WOFS, MOFS = 0, 108
NT0 = (NJ + 1) // 2  # 22
NT8 = NJ // 2        # 21
NCOL = NT0 * CHUNK   # 9152

NPG = 22             # psum groups u: j = {2u, 2u+1} (u=21: j=42 only)
NOG = 6              # output groups of 4 psum groups
VOFS = 11 * 600      # out cols [0,VOFS) = DVE (even u) groups, rest ACT

# input stages as psum-group ranges
STAGES = [(0, 2), (2, 6), (6, 13), (13, NPG)]
# output groups as psum-group ranges
OGS = [(0, 6), (6, 12), (12, 18), (18, 20), (20, NPG)]
OG_ENG = ["sync", "gpsimd", "gpsimd", "sync", "scalar"]


def _stage_cols(s):
    ulo, uhi = STAGES[s]
    return ulo * CHUNK, uhi * CHUNK


_cache = {}
LAST_RESULTS = None


def _build_nc():
    import concourse.bacc as bacc
    import concourse.tile as tile
    import concourse.mybir as mybir

    f16 = mybir.dt.float16
    f32 = mybir.dt.float32

    nc = bacc.Bacc(
        "TRN2",
        target_bir_lowering=False,
        debug=False,
        enable_asserts=False,
        num_devices=8,
    )
    # rows 0-35: T0 stream, rows 64-99: T8 stream (rows 36-63 pad so each
    # stage moves as ONE rectangle; the pad rides otherwise-idle SDMA
    # engines).
    inp_d = nc.dram_tensor("inp", [100, NCOL], f16, kind="ExternalInput").ap()
    # columns [0, VOFS) = T0 (DVE) outputs by q, rest = T8 (ACT) by q
    out_d = nc.dram_tensor("out", [M, NJ * 300], f16, kind="ExternalOutput").ap()

    with tile.TileContext(nc) as tc:
        with (
            tc.tile_pool(name="io", bufs=1) as io,
            tc.tile_pool(name="psum", bufs=4, space="PSUM") as psum,
            tc.tile_pool(name="stage", bufs=3) as stage,
        ):
            inp = io.tile([128, NCOL], f16)
            for s in range(len(STAGES)):
                a0, a1 = _stage_cols(s)
                eng = nc.sync if s == 0 else nc.gpsimd
                eng2 = nc.scalar if s == 0 else nc.gpsimd
                eng.dma_start(out=inp[0:K, a0:a1], in_=inp_d[0:K, a0:a1])
                eng2.dma_start(
                    out=inp[64:64 + K, a0:a1], in_=inp_d[64:64 + K, a0:a1]
                )

            stv = sta = None
            ogi = 0
            svc = sac = 0          # slot counters within current og tiles
            v0 = a0_ = 0           # first vidx/aidx of current og
            for u in range(NPG):
                njs = 2 if 2 * u + 1 < NJ else 1   # j = 2u (T0), 2u+1 (T8)
                pt = psum.tile([128, 2, 512], f32, tag="pt", name=f"pt{u % 4}")
                for s in range(njs):
                    j = 2 * u + s
                    base = 0 if s == 0 else 64
                    c = u * CHUNK
                    nc.tensor.matmul(
                        pt[0:M, s, 0:300],
                        lhsT=inp[base:base + K, c + WOFS:c + WOFS + M],
                        rhs=inp[base:base + K, c + MOFS:c + MOFS + 300],
                        start=True, stop=True,
                    )
                if u == OGS[ogi][0]:
                    stv = stage.tile([M, 6, 300], f16, tag="stv",
                                     name=f"stv{ogi}", bufs=3)
                    sta = stage.tile([M, 6, 300], f16, tag="sta",
                                     name=f"sta{ogi}", bufs=3)
                    svc = sac = 0
                    v0, a0_ = (u + 1) // 2, u // 2
                if u % 2 == 0:
                    nc.vector.tensor_copy(
                        out=stv[0:M, svc:svc + njs, :],
                        in_=pt[0:M, 0:njs, 0:300]
                    )
                    svc += njs
                else:
                    nc.scalar.copy(
                        out=sta[0:M, sac:sac + njs, :],
                        in_=pt[0:M, 0:njs, 0:300]
                    )
                    sac += njs
                if u == OGS[ogi][1] - 1:
                    dv = out_d[0:M, 600 * v0:600 * v0 + 300 * svc]
                    da = out_d[0:M, VOFS + 600 * a0_:
                               VOFS + 600 * a0_ + 300 * sac]
                    eng = getattr(nc, OG_ENG[ogi])
                    eng.dma_start(out=dv, in_=stv[0:M, 0:svc, :])
                    eng.dma_start(out=da, in_=sta[0:M, 0:sac, :])
                    ogi += 1
    nc.compile()
    return nc


def _get_nc():
    if "nc" not in _cache:
        _cache["nc"] = _build_nc()
    return _cache["nc"]


def _prep_inputs(observed_ipd, query_azi, query_ele, pair_vectors, freq_bins):
    obs = np.asarray(observed_ipd, np.float64).reshape(B, P, F, T)
    azi = np.asarray(query_azi, np.float64)
    ele = np.asarray(query_ele, np.float64)
    pv = np.asarray(pair_vectors, np.float64)
    fb = np.asarray(freq_bins, np.float64)

    se, ce = np.sin(ele), np.cos(ele)
    r = np.stack([se * np.cos(azi), se * np.sin(azi), ce], axis=1)  # (B,3,NQ)
    tdoa = np.einsum("pc,bcn->bpn", pv, r) / V_SOUND
    tpd = 2.0 * np.pi * tdoa[..., None] * fb  # (B,P,NQ,F)
    wtrig = np.stack([np.cos(tpd), np.sin(tpd)])  # (2,B,P,NQ,F)

    in_maps = []
    for b in range(B):
        otrig = np.stack([np.cos(obs[b]), np.sin(obs[b])])  # (2,P,F,T)
        for h in range(2):
            inp = np.zeros((100, NCOL), np.float16)
            for j in range(NJ):
                gf = FBASE[h] + 3 * j + np.arange(G)
                row = 0 if j % 2 == 0 else 64
                q = j // 2
                c = q * CHUNK
                ma = otrig[:, :, gf, :]               # (2,P,G,T)
                inp[row:row + K, c + MOFS:c + MOFS + 300] = ma.reshape(K, T)
                wsel = wtrig[:, b][:, :, :, gf]       # (2,P,NQ,G)
                wfull = np.zeros((2, P, G, NQ, G))
                for g in range(G):
                    wfull[:, :, g, :, g] = wsel[:, :, :, g]
                inp[row:row + K, c + WOFS:c + WOFS + M] = wfull.reshape(K, M)
            in_maps.append({"inp": np.ascontiguousarray(inp)})
    return in_maps


def _unscramble(o):
    """o: [108, 12900] fp16 -> V_half [NQ, FPC, T] fp32."""
    v = np.empty((NQ, FPC, T), np.float32)
    of = o.astype(np.float32)
    for j in range(NJ):
        u = j // 2
        if u % 2 == 0:
            c = 600 * (u // 2) + 300 * (j % 2)
        else:
            c = VOFS + 600 * (u // 2) + 300 * (j % 2)
        v[:, 3 * j:3 * j + G, :] = of[:, c:c + 300].reshape(NQ, G, T)
    return v


def kernel(observed_ipd, query_azi, query_ele, pair_vectors, freq_bins):
    global LAST_RESULTS
    from concourse.bass_utils import run_bass_kernel_spmd

    nc = _get_nc()
    in_maps = _prep_inputs(
        observed_ipd, query_azi, query_ele, pair_vectors, freq_bins
    )
    res = run_bass_kernel_spmd(nc, in_maps, core_ids=list(range(8)))
    LAST_RESULTS = res
    out = np.empty((B, NQ, F, T), np.float32)
    for c in range(8):
        b, h = divmod(c, 2)
        v = _unscramble(res.results[c]["out"])
        out[b, :, FBASE[h]:FBASE[h] + FPC, :] = v
    return out


# revision 5
# speedup vs baseline: 1.1650x; 1.0014x over previous
"""Trainium2 Bass kernel for CalculateDirectionFeature.

V[b,n,f,t] = sum_p cos(obs_ipd[b,p,f,t] - tpd[b,p,n,f]);
cos(a-b) = cos(a)cos(b) + sin(a)sin(b) -> matmul contracting
K = (2 trig x 6 pairs x G=3 freqs) = 36, M = 108 (36 dirs x 3 freqs),
N = 300 (time).  Host precomputes both trig factors; 8 cores =
4 batches x 2 freq halves (129 bins each, 1 overlapping bin).

PE: 64x128 two-tile mode — psum group u computes j=2u on row-tile T0
(operands at SBUF partitions 0-35) and j=2u+1 on T8 (partitions
64-99).  The split's main win is the input DMA: each stream's
partitions map onto a distinct (even/odd) half of the 16 SDMA engines.

PSUM: [128, 2banks, 512] tile per group, 4 rotating buffers (all 8
banks).  Evacuation alternates whole groups between DVE (even u) and
ACT (odd u) into separate staging tiles — exactly one reader per psum
buffer keeps the Tile sem assigner from chaining cross-engine waits,
and the 4-deep rotation rides out its conservative (+2 group) ticks.

Input: stage 0 split across the sync+scalar HWDGE rings (parallel
issue, low first-byte latency), later stages on the gpsimd SWDGE ring
(fastest for bulk DRAM->SBUF).  Output: flat [108, 12900] fp16 in
(engine-region, group, slot, t) order; middle output groups ride the
gpsimd ring, first/last ride sync (HWDGE's shorter completion receipt
shortens the final drain).  Host unscrambles and upcasts.
"""

import numpy as np

B, P, NQ, F, T = 4, 6, 36, 257, 300
V_SOUND = 343.0
G = 3
M = NQ * G           # 108
K = 2 * P * G        # 36
NJ = 43
FPC = NJ * G         # 129
FBASE = [0, F - FPC]

CHUNK = 416          # 108 wts + 300 marr + 8 pad
WOFS, MOFS = 0, 108
NT0 = (NJ + 1) // 2  # 22
NT8 = NJ // 2        # 21
NCOL = NT0 * CHUNK   # 9152

NPG = 22             # psum groups u: j = {2u, 2u+1} (u=21: j=42 only)
NOG = 6              # output groups of 4 psum groups
VOFS = 11 * 600      # out cols [0,VOFS) = DVE (even u) groups, rest ACT

# input stages as psum-group ranges
STAGES = [(0, 2), (2, 7), (7, 14), (14, NPG)]
# output groups as psum-group ranges
OGS = [(0, 6), (6, 12), (12, 18), (18, 20), (20, NPG)]
OG_ENG = ["sync", "gpsimd", "gpsimd", "sync", "scalar"]


def _stage_cols(s):
    ulo, uhi = STAGES[s]
    return ulo * CHUNK, uhi * CHUNK


_cache = {}
LAST_RESULTS = None


def _build_nc():
    import concourse.bacc as bacc
    import concourse.tile as tile
    import concourse.mybir as mybir

    f16 = mybir.dt.float16
    f32 = mybir.dt.float32

    nc = bacc.Bacc(
        "TRN2",
        target_bir_lowering=False,
        debug=False,
        enable_asserts=False,
        num_devices=8,
    )
    # rows 0-35: T0 stream, rows 64-99: T8 stream (rows 36-63 pad so each
    # stage moves as ONE rectangle; the pad rides otherwise-idle SDMA
    # engines).
    inp_d = nc.dram_tensor("inp", [100, NCOL], f16, kind="ExternalInput").ap()
    # columns [0, VOFS) = T0 (DVE) outputs by q, rest = T8 (ACT) by q
    out_d = nc.dram_tensor("out", [M, NJ * 300], f16, kind="ExternalOutput").ap()

    with tile.TileContext(nc) as tc:
        with (
            tc.tile_pool(name="io", bufs=1) as io,
            tc.tile_pool(name="psum", bufs=4, space="PSUM") as psum,
            tc.tile_pool(name="stage", bufs=3) as stage,
        ):
            inp = io.tile([128, NCOL], f16)
            for s in range(len(STAGES)):
                a0, a1 = _stage_cols(s)
                eng = nc.sync if s == 0 else nc.gpsimd
                eng2 = nc.scalar if s == 0 else nc.gpsimd
                eng.dma_start(out=inp[0:K, a0:a1], in_=inp_d[0:K, a0:a1])
                eng2.dma_start(
                    out=inp[64:64 + K, a0:a1], in_=inp_d[64:64 + K, a0:a1]
                )

            stv = sta = None
            ogi = 0
            svc = sac = 0          # slot counters within current og tiles
            v0 = a0_ = 0           # first vidx/aidx of current og
            for u in range(NPG):
                njs = 2 if 2 * u + 1 < NJ else 1   # j = 2u (T0), 2u+1 (T8)
                pt = psum.tile([128, 2, 512], f32, tag="pt", name=f"pt{u % 4}")
                for s in range(njs):
                    j = 2 * u + s
                    base = 0 if s == 0 else 64
                    c = u * CHUNK
                    nc.tensor.matmul(
                        pt[0:M, s, 0:300],
                        lhsT=inp[base:base + K, c + WOFS:c + WOFS + M],
                        rhs=inp[base:base + K, c + MOFS:c + MOFS + 300],
                        start=True, stop=True,
                    )
                if u == OGS[ogi][0]:
                    stv = stage.tile([M, 6, 300], f16, tag="stv",
                                     name=f"stv{ogi}", bufs=3)
                    sta = stage.tile([M, 6, 300], f16, tag="sta",
                                     name=f"sta{ogi}", bufs=3)
                    svc = sac = 0
                    v0, a0_ = (u + 1) // 2, u // 2
                if u % 2 == 0:
                    nc.vector.tensor_copy(
                        out=stv[0:M, svc:svc + njs, :],
                        in_=pt[0:M, 0:njs, 0:300]
                    )
                    svc += njs
                else:
                    nc.scalar.copy(
                        out=sta[0:M, sac:sac + njs, :],
                        in_=pt[0:M, 0:njs, 0:300]
                    )
                    sac += njs
                if u == OGS[ogi][1] - 1:
                    dv = out_d[0:M, 600 * v0:600 * v0 + 300 * svc]
                    da = out_d[0:M, VOFS + 600 * a0_:
                               VOFS + 600 * a0_ + 300 * sac]
                    eng = getattr(nc, OG_ENG[ogi])
                    eng.dma_start(out=dv, in_=stv[0:M, 0:svc, :])
                    eng.dma_start(out=da, in_=sta[0:M, 0:sac, :])
                    ogi += 1
    nc.compile()
    return nc


def _get_nc():
    if "nc" not in _cache:
        _cache["nc"] = _build_nc()
    return _cache["nc"]


def _prep_inputs(observed_ipd, query_azi, query_ele, pair_vectors, freq_bins):
    obs = np.asarray(observed_ipd, np.float64).reshape(B, P, F, T)
    azi = np.asarray(query_azi, np.float64)
    ele = np.asarray(query_ele, np.float64)
    pv = np.asarray(pair_vectors, np.float64)
    fb = np.asarray(freq_bins, np.float64)

    se, ce = np.sin(ele), np.cos(ele)
    r = np.stack([se * np.cos(azi), se * np.sin(azi), ce], axis=1)  # (B,3,NQ)
    tdoa = np.einsum("pc,bcn->bpn", pv, r) / V_SOUND
    tpd = 2.0 * np.pi * tdoa[..., None] * fb  # (B,P,NQ,F)
    wtrig = np.stack([np.cos(tpd), np.sin(tpd)])  # (2,B,P,NQ,F)

    in_maps = []
    for b in range(B):
        otrig = np.stack([np.cos(obs[b]), np.sin(obs[b])])  # (2,P,F,T)
        for h in range(2):
            inp = np.zeros((100, NCOL), np.float16)
            for j in range(NJ):
                gf = FBASE[h] + 3 * j + np.arange(G)
                row = 0 if j % 2 == 0 else 64
                q = j // 2
                c = q * CHUNK
                ma = otrig[:, :, gf, :]               # (2,P,G,T)
                inp[row:row + K, c + MOFS:c + MOFS + 300] = ma.reshape(K, T)
                wsel = wtrig[:, b][:, :, :, gf]       # (2,P,NQ,G)
                wfull = np.zeros((2, P, G, NQ, G))
                for g in range(G):
                    wfull[:, :, g, :, g] = wsel[:, :, :, g]
                inp[row:row + K, c + WOFS:c + WOFS + M] = wfull.reshape(K, M)
            in_maps.append({"inp": np.ascontiguousarray(inp)})
    return in_maps


def _unscramble(o):
    """o: [108, 12900] fp16 -> V_half [NQ, FPC, T] fp32."""
    v = np.empty((NQ, FPC, T), np.float32)
    of = o.astype(np.float32)
    for j in range(NJ):
        u = j // 2
        if u % 2 == 0:
            c = 600 * (u // 2) + 300 * (j % 2)
        else:
            c = VOFS + 600 * (u // 2) + 300 * (j % 2)
        v[:, 3 * j:3 * j + G, :] = of[:, c:c + 300].reshape(NQ, G, T)
    return v


def kernel(observed_ipd, query_azi, query_ele, pair_vectors, freq_bins):
    global LAST_RESULTS
    from concourse.bass_utils import run_bass_kernel_spmd

    nc = _get_nc()
    in_maps = _prep_inputs(
        observed_ipd, query_azi, query_ele, pair_vectors, freq_bins
    )
    res = run_bass_kernel_spmd(nc, in_maps, core_ids=list(range(8)))
    LAST_RESULTS = res
    out = np.empty((B, NQ, F, T), np.float32)
    for c in range(8):
        b, h = divmod(c, 2)
        v = _unscramble(res.results[c]["out"])
        out[b, :, FBASE[h]:FBASE[h] + FPC, :] = v
    return out
